# revision 18
# baseline (speedup 1.0000x reference)
"""Cross-attention layer on 8 Trainium2 NeuronCores (Bass/Tile SPMD).

Sharding: tensor-parallel over heads. Each core owns 4 of the 32 heads:
it projects Q^T/K^T/V for its heads (bf16 matmuls, fp32 accumulate),
runs masked softmax attention in transposed layout (scores^T so the
softmax v-reduction is a PE ones-matmul and no attn transpose is ever
needed), then a split AllToAll redistributes ctx^T from head-sharded to
token-sharded so every core runs the output projection + residual +
LayerNorm for its own 256-token slice. Host concatenates the 8 slices.

Schedule highlights (~1.4x over the naive phase-serial version):
- all DRAM tensors pre-tiled on host so every DMA is a contiguous
  per-partition run; first weight/activation loads split in halves so
  the PE starts ~10us into the kernel
- K and V projections share one vis xT load per token quarter
- attention loops head-major; the AllToAll is split in two: heads 0-1
  fire mid-attention (collective hidden under heads 2-3), and the
  first two O-proj chunk chains issue their collective-1-only matmuls
  early so the PE keeps working while collective 2 is in flight
- Wo chunks prefetch during attention; softmax normalize orders the
  ctx matmuls before the reciprocal broadcast so the PE never waits
  on the DVE reciprocal
- LayerNorm fused: mean accumulated on the PSUM->SBUF drain pass,
  (x-mu)^2 via the Square activation bias port, bf16 DVE passes

Numerics: matmul inputs bf16, accumulation fp32, softmax without
max-subtraction (scores ~N(0,1)), mask folded into the exp bias,
1/sqrt(hd) folded into Wq on host, bv folded into an effective bo on
host, LN stats in fp32, LN elementwise tail + output in bf16
(rel err ~6e-3 of output scale vs the fp32 reference).
"""
import sys

sys.path.insert(0, "/opt/trn_rl_repo")

import numpy as np
import ml_dtypes

import concourse.bacc as bacc
import concourse.mybir as mybir
import concourse.tile as tile
from concourse.bass_utils import run_bass_kernel_spmd

BF16 = ml_dtypes.bfloat16

NCORES = 8
P = 128            # partitions / head dim / k-tile
H = 4096
KT = H // P        # 32 k-tiles along any H contraction
NH = 32
NHL = NH // NCORES  # 4 local heads
CW = NHL * P       # 512 local c-columns
B = 2
LB = 1024          # tokens per batch
L2 = B * LB        # 2048 total tokens
TL = L2 // NCORES  # 256 tokens per core after A2A
QW = 512           # token-quarter width in phase A
NQ = L2 // QW      # 4
NVT = L2 // P      # 16 v tiles total (8 per batch)
MC = 8             # Wo column chunks
MCW = H // MC      # 512
MSK = -1e30

_CACHE = {}

F32 = mybir.dt.float32
BF = mybir.dt.bfloat16


def _build(debug=False, reps=1, do_b=True, do_a2a=True, do_c=True,
           serialize=False, attnt_bufs=3, rcprep_bufs=1):
    nc = bacc.Bacc("TRN2", target_bir_lowering=False, debug=False,
                   num_devices=NCORES)

    # activations pre-tiled [p, q, kt, l]; weights [p, kt, c]; wo [p, mc, ct, m]
    hidT_d = nc.dram_tensor("hidT", [P, NQ * KT * QW], BF, kind="ExternalInput")
    visT_d = nc.dram_tensor("visT", [P, NQ * KT * QW], BF, kind="ExternalInput")
    wqT_d = nc.dram_tensor("wqT", [P, KT * CW], BF, kind="ExternalInput")
    wkT_d = nc.dram_tensor("wkT", [P, KT * CW], BF, kind="ExternalInput")
    wvT_d = nc.dram_tensor("wvT", [P, KT * CW], BF, kind="ExternalInput")
    woT_d = nc.dram_tensor("woT", [P, MC * KT * MCW], BF, kind="ExternalInput")
    bqT_d = nc.dram_tensor("bqT", [P, NHL], F32, kind="ExternalInput")
    bkT_d = nc.dram_tensor("bkT", [P, NHL], F32, kind="ExternalInput")
    mskb_d = nc.dram_tensor("mskb", [P, B * 8], F32, kind="ExternalInput")
    hb_d = nc.dram_tensor("hb", [TL, H], F32, kind="ExternalInput")
    g_d = nc.dram_tensor("g", [P, H], BF, kind="ExternalInput")
    bta_d = nc.dram_tensor("bta", [P, H], BF, kind="ExternalInput")
    out_d = nc.dram_tensor("out", [TL, H], BF, kind="ExternalOutput")

    HKT = KT // 2          # kt half (16) for chunked weight/activation loads
    HCW = HKT * CW         # weight cols per half tile
    HQW = HKT * QW         # activation cols per half tile

    with tile.TileContext(nc) as tc:
        for _rep in range(reps):
            if serialize and _rep:
                tc.strict_bb_all_engine_barrier()
            with tc.tile_pool(name="persist", bufs=1) as pers, \
                 tc.tile_pool(name="dram", bufs=1, space="DRAM") as dram:

                # one 5-slot pool of 16KB buffers: qT/kT/v live through A+B;
                # qT's buffer is reused in place for ctx^T; in phase C the
                # slots recycle as x0/x1/hb0/hb1/o0/o1.
                big = tc.alloc_tile_pool(name="big", bufs=1)
                qT_sb = big.tile([P, NHL * L2], BF, tag="big", bufs=5)
                kT_sb = big.tile([P, NHL * L2], BF, tag="big", bufs=5)
                v_sb = big.tile([P, NVT * CW], BF, tag="big", bufs=5)
                ctxT_sb = qT_sb  # in-place: ctx^T overwrites Q^T per block
                bqT_sb = pers.tile([P, NHL], F32)
                bkT_sb = pers.tile([P, NHL], F32)
                mskb_sb = pers.tile([P, B * 8], F32)
                ones_bf = pers.tile([P, 1], BF)
                ones_f32 = pers.tile([1, P], F32)
                eps_sb = pers.tile([P, 1], F32)
                nc.vector.memset(ones_bf[:], 1.0)
                nc.vector.memset(ones_f32[:], 1.0)
                nc.vector.memset(eps_sb[:], 1e-5)

                # ---------------- Phase A: Q^T, K^T, V projections ----------
                pa = tc.alloc_tile_pool(name="phaseA", bufs=1)
                with tc.tile_pool(name="psA", bufs=6, space="PSUM") as psA:
                    def load_w_half(dram_t, s, name):
                        w = pa.tile([P, HCW], BF, tag="w", bufs=4, name=name)
                        nc.sync.dma_start(
                            out=w[:], in_=dram_t[:, s * HCW:(s + 1) * HCW])
                        return w

                    def load_x_half(dram_t, q, s, name):
                        x = pa.tile([P, HQW], BF, tag="xT", bufs=3, name=name)
                        nc.sync.dma_start(
                            out=x[:],
                            in_=dram_t[:, (2 * q + s) * HQW:
                                       (2 * q + s + 1) * HQW])
                        return x

                    wq = [load_w_half(wqT_d, 0, "wqA")]
                    xh = [load_x_half(hidT_d, 0, 0, "xh0A")]
                    wq.append(load_w_half(wqT_d, 1, "wqB"))
                    xh.append(load_x_half(hidT_d, 0, 1, "xh0B"))
                    nc.sync.dma_start(out=bqT_sb[:], in_=bqT_d[:])
                    nc.sync.dma_start(out=bkT_sb[:], in_=bkT_d[:])
                    nc.sync.dma_start(out=mskb_sb[:], in_=mskb_d[:])
                    wk = [load_w_half(wkT_d, 0, "wkA"),
                          load_w_half(wkT_d, 1, "wkB")]

                    def qk_mms(xp, w, b_sb, dst_sb, q):
                        for h in range(NHL):
                            ps = psA.tile([P, QW], F32, tag="psA")
                            for kt in range(KT):
                                s, k = divmod(kt, HKT)
                                nc.tensor.matmul(
                                    ps[:],
                                    w[s][:, k * CW + h * P: k * CW + (h + 1) * P],
                                    xp[s][:, k * QW:(k + 1) * QW],
                                    start=(kt == 0), stop=(kt == KT - 1))
                            nc.vector.tensor_scalar_add(
                                dst_sb[:, h * L2 + q * QW: h * L2 + (q + 1) * QW],
                                ps[:], b_sb[:, h:h + 1])

                    # Q projection over 4 hid quarters
                    for q in range(NQ):
                        if q > 0:
                            xh = [load_x_half(hidT_d, q, s, f"xh{q}{s}")
                                  for s in range(2)]
                        qk_mms(xh, wq, bqT_sb, qT_sb, q)

                    # K + V share one vis quarter load
                    wv = [load_w_half(wvT_d, 0, "wvA"),
                          load_w_half(wvT_d, 1, "wvB")]
                    for q in range(NQ):
                        xv = [load_x_half(visT_d, q, s, f"xv{q}{s}")
                              for s in range(2)]
                        qk_mms(xv, wk, bkT_sb, kT_sb, q)
                        for vt in range(4):
                            g_vt = q * 4 + vt
                            ps = psA.tile([P, CW], F32, tag="psA")
                            for kt in range(KT):
                                s, k = divmod(kt, HKT)
                                nc.tensor.matmul(
                                    ps[:],
                                    xv[s][:, k * QW + vt * P:
                                          k * QW + (vt + 1) * P],
                                    wv[s][:, k * CW:(k + 1) * CW],
                                    start=(kt == 0), stop=(kt == KT - 1))
                            nc.scalar.copy(
                                out=v_sb[:, g_vt * CW:(g_vt + 1) * CW], in_=ps[:])
                pa.release()

                # -------- Phase C prefetch (runs during attention) ----------
                pfc = tc.alloc_tile_pool(name="prefetchC", bufs=1)
                octxT = pfc.tile([P, KT * TL], BF)   # [hd,(i,ct,l)] full ctx^T
                g_sb = pfc.tile([P, H], BF)
                bta_sb = pfc.tile([P, H], BF)
                wo_sbs = []
                for mc in range(2):
                    wo_sb = pfc.tile([P, KT * MCW], BF, tag="wo", bufs=2,
                                     name=f"wo{mc}")
                    wo_sbs.append(wo_sb)
                    nc.sync.dma_start(
                        out=wo_sb[:],
                        in_=woT_d[:, mc * (KT * MCW):(mc + 1) * (KT * MCW)])

                # ---------------- Phase B: attention, head-major ------------
                a2a_in = [dram.tile([NCORES, CW // 2, TL], BF,
                                    name=f"a2a_in{k}") for k in range(2)]
                a2a_out = [dram.tile([NCORES, CW // 2, TL], BF,
                                     name=f"a2a_out{k}") for k in range(2)]

                with tc.tile_pool(name="phaseB", bufs=1) as pb, \
                     tc.tile_pool(name="psB", bufs=1, space="PSUM") as psB:
                    for h in range(NHL if do_b else 0):
                        for b in range(B):
                            for lh in range(2):
                                qcol = h * L2 + b * LB + lh * QW
                                attnT = pb.tile([P, 8 * QW], BF, tag="attnT",
                                                bufs=attnt_bufs)
                                rs_ps = psB.tile([1, QW], F32, tag="rs", bufs=2)
                                for vb in range(8):
                                    sc_ps = psB.tile([P, QW], F32, tag="sc",
                                                     bufs=3)
                                    nc.tensor.matmul(
                                        sc_ps[:],
                                        kT_sb[:, h * L2 + b * LB + vb * P:
                                              h * L2 + b * LB + (vb + 1) * P],
                                        qT_sb[:, qcol: qcol + QW],
                                        start=True, stop=True)
                                    mcol = b * 8 + vb
                                    nc.scalar.activation(
                                        attnT[:, vb * QW:(vb + 1) * QW], sc_ps[:],
                                        mybir.ActivationFunctionType.Exp,
                                        bias=mskb_sb[:, mcol:mcol + 1], scale=1.0)
                                    nc.tensor.matmul(
                                        rs_ps[:], ones_bf[:],
                                        attnT[:, vb * QW:(vb + 1) * QW],
                                        start=(vb == 0), stop=(vb == 7))
                                rcp_sb = pb.tile([1, QW], F32, tag="rcp", bufs=2)
                                nc.vector.reciprocal(rcp_sb[:], rs_ps[:])
                                ctx_ps = psB.tile([P, QW], F32, tag="ctx",
                                                  bufs=2)
                                for vb in range(8):
                                    nc.tensor.matmul(
                                        ctx_ps[:],
                                        v_sb[:, (b * 8 + vb) * CW + h * P:
                                             (b * 8 + vb) * CW + (h + 1) * P],
                                        attnT[:, vb * QW:(vb + 1) * QW],
                                        start=(vb == 0), stop=(vb == 7))
                                rcp_ps = psB.tile([P, QW], F32, tag="rcpp",
                                                  bufs=1)
                                nc.tensor.matmul(rcp_ps[:], ones_f32[:],
                                                 rcp_sb[:], start=True, stop=True)
                                rcp_rep = pb.tile([P, QW], F32, tag="rcprep",
                                                  bufs=rcprep_bufs)
                                nc.scalar.copy(out=rcp_rep[:], in_=rcp_ps[:])
                                nc.vector.tensor_tensor(
                                    out=ctxT_sb[:, qcol: qcol + QW],
                                    in0=ctx_ps[:], in1=rcp_rep[:],
                                    op=mybir.AluOpType.mult)
                        # after heads {0,1} done, fire first half A2A
                        if do_a2a and h == 1:
                            for hh in range(2):
                                nc.sync.dma_start(
                                    out=a2a_in[0][:, hh * P:(hh + 1) * P, :]
                                        .rearrange("j p l -> p j l"),
                                    in_=ctxT_sb[:, hh * L2:(hh + 1) * L2]
                                        .rearrange("p (j l) -> p j l", j=NCORES))
                            nc.gpsimd.collective_compute(
                                "AllToAll", mybir.AluOpType.bypass,
                                replica_groups=[list(range(NCORES))],
                                ins=[a2a_in[0][:]], outs=[a2a_out[0][:]])
                            for i in range(NCORES):
                                nc.sync.dma_start(
                                    out=octxT[:, (i * NHL) * TL:
                                              (i * NHL + 2) * TL]
                                        .rearrange("p (ct l) -> p ct l", ct=2),
                                    in_=a2a_out[0][i]
                                        .rearrange("(ct p) l -> p ct l", p=P))

                if do_a2a:
                    for hh in range(2):
                        nc.sync.dma_start(
                            out=a2a_in[1][:, hh * P:(hh + 1) * P, :]
                                .rearrange("j p l -> p j l"),
                            in_=ctxT_sb[:, (2 + hh) * L2:(3 + hh) * L2]
                                .rearrange("p (j l) -> p j l", j=NCORES))
                    nc.gpsimd.collective_compute(
                        "AllToAll", mybir.AluOpType.bypass,
                        replica_groups=[list(range(NCORES))],
                        ins=[a2a_in[1][:]], outs=[a2a_out[1][:]])

                # ---------------- Phase C: O-proj, residual + LN ------------
                with tc.tile_pool(name="phaseC", bufs=2) as pc, \
                     tc.tile_pool(name="psC", bufs=4, space="PSUM") as psC:
                    # octxT column layout: g = i*NHL + ct, tile g at cols
                    # [g*TL, (g+1)*TL). Half-0 loads were issued right after
                    # collective #1 inside phase B; load half 1 here.
                    for i in range(NCORES if (do_c and do_a2a) else 0):
                        nc.sync.dma_start(
                            out=octxT[:, (i * NHL + 2) * TL:
                                      (i * NHL + 4) * TL]
                                .rearrange("p (ct l) -> p ct l", ct=2),
                            in_=a2a_out[1][i]
                                .rearrange("(ct p) l -> p ct l", p=P))
                    half0 = [i * NHL + ct for ct in (0, 1)
                             for i in range(NCORES)]
                    half1 = [i * NHL + ct for ct in (2, 3)
                             for i in range(NCORES)]
                    g_order = half0 + half1

                    def oproj_mms(po, wo_sb, lt, gs, start, stop):
                        for g in gs:
                            nc.tensor.matmul(
                                po[:],
                                octxT[:, g * TL + lt * P:
                                      g * TL + (lt + 1) * P],
                                wo_sb[:, g * MCW:(g + 1) * MCW],
                                start=(start and g == gs[0]),
                                stop=(stop and g == gs[-1]))

                    if do_c:
                        x_sb = [big.tile([P, H], BF, tag="big", bufs=5,
                                         name=f"x_sb{lt}") for lt in range(2)]
                        hb_sb = [big.tile([P, H], F32, tag="big", bufs=5,
                                          name=f"hb_sb{lt}") for lt in range(2)]
                        ms8 = [pc.tile([P, MC], F32, tag="ms8",
                                       name=f"ms8{lt}") for lt in range(2)]
                        nc.sync.dma_start(out=g_sb[:], in_=g_d[:])
                        nc.sync.dma_start(out=bta_sb[:], in_=bta_d[:])
                        for lt in range(2):
                            nc.sync.dma_start(
                                out=hb_sb[lt][:],
                                in_=hb_d[lt * P:(lt + 1) * P, :])

                        def drain(po, mc, lt):
                            # x = po + hb, accumulating row-sums for mean
                            nc.vector.scalar_tensor_tensor(
                                out=x_sb[lt][:, mc * MCW:(mc + 1) * MCW],
                                in0=po[:], scalar=1.0,
                                in1=hb_sb[lt][:, mc * MCW:(mc + 1) * MCW],
                                op0=mybir.AluOpType.mult,
                                op1=mybir.AluOpType.add,
                                accum_out=ms8[lt][:, mc:mc + 1])

                        # stage 1: mc 0-1 chains issue their half-0 matmuls
                        # first -- they only need collective #1, so the PE
                        # works while collective #2 is still in flight.
                        po_s1 = {}
                        for mc in range(2):
                            for lt in range(2):
                                po = psC.tile([P, MCW], F32, tag="po",
                                              name=f"po{mc}{lt}")
                                po_s1[mc, lt] = po
                                oproj_mms(po, wo_sbs[mc], lt, half0,
                                          True, False)
                        for mc in range(2):
                            for lt in range(2):
                                po = po_s1[mc, lt]
                                oproj_mms(po, wo_sbs[mc], lt, half1,
                                          False, True)
                                drain(po, mc, lt)
                        # stage 2: remaining chunks, full chains
                        for mc in range(2, MC):
                            wo_sb = pfc.tile([P, KT * MCW], BF, tag="wo",
                                             bufs=2, name=f"wo{mc}")
                            nc.sync.dma_start(
                                out=wo_sb[:],
                                in_=woT_d[:, mc * (KT * MCW):
                                          (mc + 1) * (KT * MCW)])
                            for lt in range(2):
                                po = psC.tile([P, MCW], F32, tag="po")
                                oproj_mms(po, wo_sb, lt, g_order, True, True)
                                drain(po, mc, lt)

                        for lt in range(2):
                            x = x_sb[lt]
                            o_sb = big.tile([P, H], BF, tag="big", bufs=5,
                                            name=f"o_sb{lt}")
                            musum = pc.tile([P, 1], F32, tag="musum")
                            nc.vector.tensor_reduce(
                                musum[:], ms8[lt][:], mybir.AxisListType.X,
                                mybir.AluOpType.add)
                            mu_neg = pc.tile([P, 1], F32, tag="mu")
                            nc.scalar.mul(mu_neg[:], musum[:], -1.0 / H)
                            ssq = pc.tile([P, 1], F32, tag="ssq")
                            nc.scalar.activation(
                                o_sb[:], x[:],
                                mybir.ActivationFunctionType.Square,
                                bias=mu_neg[:], scale=1.0, accum_out=ssq[:])
                            std = pc.tile([P, 1], F32, tag="std")
                            nc.scalar.activation(
                                std[:], ssq[:],
                                mybir.ActivationFunctionType.Sqrt,
                                bias=eps_sb[:], scale=1.0 / H)
                            rstd = pc.tile([P, 1], F32, tag="rstd")
                            nc.vector.reciprocal(rstd[:], std[:])
                            nc.vector.tensor_scalar(
                                out=x[:], in0=x[:], scalar1=mu_neg[:],
                                scalar2=rstd[:], op0=mybir.AluOpType.add,
                                op1=mybir.AluOpType.mult)
                            nc.vector.scalar_tensor_tensor(
                                out=o_sb[:], in0=x[:], scalar=1.0,
                                in1=g_sb[:], op0=mybir.AluOpType.mult,
                                op1=mybir.AluOpType.mult)
                            nc.vector.tensor_tensor(
                                out=o_sb[:], in0=o_sb[:], in1=bta_sb[:],
                                op=mybir.AluOpType.add)
                            nc.sync.dma_start(out=out_d[lt * P:(lt + 1) * P, :],
                                              in_=o_sb[:])
                pfc.release()
                big.release()

    nc.compile()
    return nc


def _prep_inputs(hidden_states, vision_features, attention_mask,
                 Wq, bq, Wk, bk, Wv, bv, Wo, bo, ln_g, ln_b):
    f = np.asarray
    hs = f(hidden_states, dtype=np.float32).reshape(L2, H)
    vf = f(vision_features, dtype=np.float32).reshape(L2, H)
    am = f(attention_mask)
    Wq, bq = f(Wq, dtype=np.float32), f(bq, dtype=np.float32)
    Wk, bk = f(Wk, dtype=np.float32), f(bk, dtype=np.float32)
    Wv, bv = f(Wv, dtype=np.float32), f(bv, dtype=np.float32)
    Wo, bo = f(Wo, dtype=np.float32), f(bo, dtype=np.float32)
    ln_g, ln_b = f(ln_g, dtype=np.float32), f(ln_b, dtype=np.float32)

    s = 1.0 / np.sqrt(P)

    def tile_act(x):  # [L2, H] -> [P, (q, kt, l)] with x[q*QW+l, kt*P+p]
        t = x.reshape(NQ, QW, KT, P).transpose(3, 0, 2, 1)
        return np.ascontiguousarray(t.reshape(P, NQ * KT * QW)).astype(BF16)

    def tile_w(wT):  # [H, CW] -> [P, (kt, c)]
        t = wT.reshape(KT, P, CW).transpose(1, 0, 2)
        return np.ascontiguousarray(t.reshape(P, KT * CW)).astype(BF16)

    hidT = tile_act(hs)
    visT = tile_act(vf)
    WoT = np.ascontiguousarray(Wo.T)  # [H, H]
    wo_t = np.ascontiguousarray(
        WoT.reshape(KT, P, MC, MCW).transpose(1, 2, 0, 3)
        .reshape(P, MC * KT * MCW)).astype(BF16)
    mb = np.where(am != 0, 0.0, MSK).astype(np.float32)          # (B, LB)
    mskb = np.ascontiguousarray(
        mb.reshape(B, 8, P).transpose(2, 0, 1).reshape(P, B * 8))
    bo_eff = bo + Wo @ bv
    g_rep = np.ascontiguousarray(np.broadcast_to(ln_g, (P, H))).astype(BF16)
    b_rep = np.ascontiguousarray(np.broadcast_to(ln_b, (P, H))).astype(BF16)

    in_maps = []
    for c in range(NCORES):
        sl = slice(c * CW, (c + 1) * CW)
        in_maps.append({
            "hidT": hidT,
            "visT": visT,
            "wqT": tile_w(np.ascontiguousarray((Wq[sl] * s).T)),
            "wkT": tile_w(np.ascontiguousarray(Wk[sl].T)),
            "wvT": tile_w(np.ascontiguousarray(Wv[sl].T)),
            "woT": wo_t,
            "bqT": np.ascontiguousarray((bq[sl] * s).reshape(NHL, P).T),
            "bkT": np.ascontiguousarray(bk[sl].reshape(NHL, P).T),
            "mskb": mskb,
            "hb": np.ascontiguousarray(hs[c * TL:(c + 1) * TL] + bo_eff),
            "g": g_rep,
            "bta": b_rep,
        })
    return in_maps


def kernel(**inputs) -> np.ndarray:
    inputs.pop("_debug", None)
    if "main" not in _CACHE:
        _CACHE["main"] = _build(debug=False)
    nc = _CACHE["main"]
    in_maps = _prep_inputs(**inputs)
    res = run_bass_kernel_spmd(nc, in_maps, list(range(NCORES)))
    out = np.concatenate([res.results[c]["out"] for c in range(NCORES)], axis=0)
    return out.reshape(B, LB, H).astype(np.float32)


# revision 20
# speedup vs baseline: 1.1051x; 1.1051x over previous
"""Cross-attention layer on 8 Trainium2 NeuronCores (Bass/Tile SPMD).

Sharding: tensor-parallel over heads. Each core owns 4 of the 32 heads:
it projects Q^T/K^T/V for its heads (bf16 matmuls, fp32 accumulate),
runs masked softmax attention in transposed layout (scores^T so the
softmax v-reduction is a PE ones-matmul and no attn transpose is ever
needed), then a split AllToAll redistributes ctx^T from head-sharded to
token-sharded so every core runs the output projection + residual +
LayerNorm for its own 256-token slice. Host concatenates the 8 slices.

Schedule highlights (~1.4x over the naive phase-serial version):
- all DRAM tensors pre-tiled on host so every DMA is a contiguous
  per-partition run; first weight/activation loads split in halves so
  the PE starts ~10us into the kernel
- K and V projections share one vis xT load per token quarter
- attention loops head-major; the AllToAll is split in two: heads 0-1
  fire mid-attention (collective hidden under heads 2-3), and the
  first two O-proj chunk chains issue their collective-1-only matmuls
  early so the PE keeps working while collective 2 is in flight
- Wo chunks prefetch during attention; softmax normalize orders the
  ctx matmuls before the reciprocal broadcast so the PE never waits
  on the DVE reciprocal
- LayerNorm fused: mean accumulated on the PSUM->SBUF drain pass,
  (x-mu)^2 via the Square activation bias port, bf16 DVE passes

Numerics: matmul inputs bf16, accumulation fp32, softmax without
max-subtraction (scores ~N(0,1)), mask folded into the exp bias,
1/sqrt(hd) folded into Wq on host, bv folded into an effective bo on
host, LN stats in fp32, LN elementwise tail + output in bf16
(rel err ~6e-3 of output scale vs the fp32 reference).
"""
import sys

sys.path.insert(0, "/opt/trn_rl_repo")

import numpy as np
import ml_dtypes

import concourse.bacc as bacc
import concourse.mybir as mybir
import concourse.tile as tile
from concourse.bass_utils import run_bass_kernel_spmd

BF16 = ml_dtypes.bfloat16

NCORES = 8
P = 128            # partitions / head dim / k-tile
H = 4096
KT = H // P        # 32 k-tiles along any H contraction
NH = 32
NHL = NH // NCORES  # 4 local heads
CW = NHL * P       # 512 local c-columns
B = 2
LB = 1024          # tokens per batch
L2 = B * LB        # 2048 total tokens
TL = L2 // NCORES  # 256 tokens per core after A2A
QW = 512           # token-quarter width in phase A
NQ = L2 // QW      # 4
NVT = L2 // P      # 16 v tiles total (8 per batch)
MC = 8             # Wo column chunks
MCW = H // MC      # 512
MSK = -1e30

_CACHE = {}

F32 = mybir.dt.float32
BF = mybir.dt.bfloat16


def _build(debug=False, reps=1, do_b=True, do_a2a=True, do_c=True,
           serialize=False, attnt_bufs=3, rcprep_bufs=1):
    nc = bacc.Bacc("TRN2", target_bir_lowering=False, debug=False,
                   num_devices=NCORES)

    # activations pre-tiled [p, q, kt, l]; weights [p, kt, c]; wo [p, mc, ct, m]
    hidT_d = nc.dram_tensor("hidT", [P, NQ * KT * QW], BF, kind="ExternalInput")
    visT_d = nc.dram_tensor("visT", [P, NQ * KT * QW], BF, kind="ExternalInput")
    wqT_d = nc.dram_tensor("wqT", [P, KT * CW], BF, kind="ExternalInput")
    wkT_d = nc.dram_tensor("wkT", [P, KT * CW], BF, kind="ExternalInput")
    wvT_d = nc.dram_tensor("wvT", [P, KT * CW], BF, kind="ExternalInput")
    woT_d = nc.dram_tensor("woT", [P, MC * KT * MCW], BF, kind="ExternalInput")
    bqT_d = nc.dram_tensor("bqT", [P, NHL], F32, kind="ExternalInput")
    bkT_d = nc.dram_tensor("bkT", [P, NHL], F32, kind="ExternalInput")
    mskb_d = nc.dram_tensor("mskb", [P, B * 8], F32, kind="ExternalInput")
    hb_d = nc.dram_tensor("hb", [TL, H], F32, kind="ExternalInput")
    g_d = nc.dram_tensor("g", [P, H], BF, kind="ExternalInput")
    bta_d = nc.dram_tensor("bta", [P, H], BF, kind="ExternalInput")
    out_d = nc.dram_tensor("out", [TL, H], BF, kind="ExternalOutput")

    HKT = KT // 2          # kt half (16) for chunked weight/activation loads
    HCW = HKT * CW         # weight cols per half tile
    HQW = HKT * QW         # activation cols per half tile

    with tile.TileContext(nc) as tc:
        for _rep in range(reps):
            if serialize and _rep:
                tc.strict_bb_all_engine_barrier()
            with tc.tile_pool(name="persist", bufs=1) as pers, \
                 tc.tile_pool(name="dram", bufs=1, space="DRAM") as dram:

                # one 5-slot pool of 16KB buffers: qT/kT/v live through A+B;
                # qT's buffer is reused in place for ctx^T; in phase C the
                # slots recycle as x0/x1/hb0/hb1/o0/o1.
                big = tc.alloc_tile_pool(name="big", bufs=1)
                qT_sb = big.tile([P, NHL * L2], BF, tag="big", bufs=5)
                kT_sb = big.tile([P, NHL * L2], BF, tag="big", bufs=5)
                v_sb = big.tile([P, NVT * CW], BF, tag="big", bufs=5)
                ctxT_sb = qT_sb  # in-place: ctx^T overwrites Q^T per block
                bqT_sb = pers.tile([P, NHL], F32)
                bkT_sb = pers.tile([P, NHL], F32)
                mskb_sb = pers.tile([P, B * 8], F32)
                ones_bf = pers.tile([P, 1], BF)
                ones_f32 = pers.tile([1, P], F32)
                eps_sb = pers.tile([P, 1], F32)
                nc.vector.memset(ones_bf[:], 1.0)
                nc.vector.memset(ones_f32[:], 1.0)
                nc.vector.memset(eps_sb[:], 1e-5)

                # ---------------- Phase A: Q^T, K^T, V projections ----------
                pa = tc.alloc_tile_pool(name="phaseA", bufs=1)
                with tc.tile_pool(name="psA", bufs=6, space="PSUM") as psA:
                    def load_w_half(dram_t, s, name):
                        w = pa.tile([P, HCW], BF, tag="w", bufs=4, name=name)
                        nc.sync.dma_start(
                            out=w[:], in_=dram_t[:, s * HCW:(s + 1) * HCW])
                        return w

                    def load_x_half(dram_t, q, s, name):
                        x = pa.tile([P, HQW], BF, tag="xT", bufs=3, name=name)
                        nc.sync.dma_start(
                            out=x[:],
                            in_=dram_t[:, (2 * q + s) * HQW:
                                       (2 * q + s + 1) * HQW])
                        return x

                    wq = [load_w_half(wqT_d, 0, "wqA")]
                    xh = [load_x_half(hidT_d, 0, 0, "xh0A")]
                    wq.append(load_w_half(wqT_d, 1, "wqB"))
                    xh.append(load_x_half(hidT_d, 0, 1, "xh0B"))
                    nc.sync.dma_start(out=bqT_sb[:], in_=bqT_d[:])
                    nc.sync.dma_start(out=bkT_sb[:], in_=bkT_d[:])
                    nc.sync.dma_start(out=mskb_sb[:], in_=mskb_d[:])
                    wk = [load_w_half(wkT_d, 0, "wkA"),
                          load_w_half(wkT_d, 1, "wkB")]

                    def qk_mms(xp, w, b_sb, dst_sb, q):
                        for h in range(NHL):
                            ps = psA.tile([P, QW], F32, tag="psA")
                            for kt in range(KT):
                                s, k = divmod(kt, HKT)
                                nc.tensor.matmul(
                                    ps[:],
                                    w[s][:, k * CW + h * P: k * CW + (h + 1) * P],
                                    xp[s][:, k * QW:(k + 1) * QW],
                                    start=(kt == 0), stop=(kt == KT - 1))
                            nc.vector.tensor_scalar_add(
                                dst_sb[:, h * L2 + q * QW: h * L2 + (q + 1) * QW],
                                ps[:], b_sb[:, h:h + 1])

                    # Q projection over 4 hid quarters
                    for q in range(NQ):
                        if q > 0:
                            xh = [load_x_half(hidT_d, q, s, f"xh{q}{s}")
                                  for s in range(2)]
                        qk_mms(xh, wq, bqT_sb, qT_sb, q)

                    # K + V share one vis quarter load
                    wv = [load_w_half(wvT_d, 0, "wvA"),
                          load_w_half(wvT_d, 1, "wvB")]
                    for q in range(NQ):
                        xv = [load_x_half(visT_d, q, s, f"xv{q}{s}")
                              for s in range(2)]
                        qk_mms(xv, wk, bkT_sb, kT_sb, q)
                        for vt in range(4):
                            g_vt = q * 4 + vt
                            ps = psA.tile([P, CW], F32, tag="psA")
                            for kt in range(KT):
                                s, k = divmod(kt, HKT)
                                nc.tensor.matmul(
                                    ps[:],
                                    xv[s][:, k * QW + vt * P:
                                          k * QW + (vt + 1) * P],
                                    wv[s][:, k * CW:(k + 1) * CW],
                                    start=(kt == 0), stop=(kt == KT - 1))
                            nc.scalar.copy(
                                out=v_sb[:, g_vt * CW:(g_vt + 1) * CW], in_=ps[:])
                pa.release()

                # -------- Phase C prefetch (runs during attention) ----------
                pfc = tc.alloc_tile_pool(name="prefetchC", bufs=1)
                octxT = pfc.tile([P, KT * TL], BF)   # [hd,(i,ct,l)] full ctx^T
                g_sb = pfc.tile([P, H], BF)
                bta_sb = pfc.tile([P, H], BF)
                wo_sbs = []
                for mc in range(2):
                    wo_sb = pfc.tile([P, KT * MCW], BF, tag="wo", bufs=2,
                                     name=f"wo{mc}")
                    wo_sbs.append(wo_sb)
                    nc.sync.dma_start(
                        out=wo_sb[:],
                        in_=woT_d[:, mc * (KT * MCW):(mc + 1) * (KT * MCW)])

                # ---------------- Phase B: attention, head-major ------------
                a2a_in = [dram.tile([NCORES, CW // 2, TL], BF,
                                    name=f"a2a_in{k}") for k in range(2)]
                a2a_out = [dram.tile([NCORES, CW // 2, TL], BF,
                                     name=f"a2a_out{k}") for k in range(2)]

                with tc.tile_pool(name="phaseB", bufs=1) as pb, \
                     tc.tile_pool(name="psB", bufs=1, space="PSUM") as psB:
                    for h in range(NHL if do_b else 0):
                        for b in range(B):
                            for lh in range(2):
                                qcol = h * L2 + b * LB + lh * QW
                                attnT = pb.tile([P, 8 * QW], BF, tag="attnT",
                                                bufs=attnt_bufs)
                                rs_ps = psB.tile([1, QW], F32, tag="rs", bufs=2)
                                for vb in range(8):
                                    sc_ps = psB.tile([P, QW], F32, tag="sc",
                                                     bufs=3)
                                    nc.tensor.matmul(
                                        sc_ps[:],
                                        kT_sb[:, h * L2 + b * LB + vb * P:
                                              h * L2 + b * LB + (vb + 1) * P],
                                        qT_sb[:, qcol: qcol + QW],
                                        start=True, stop=True)
                                    mcol = b * 8 + vb
                                    nc.scalar.activation(
                                        attnT[:, vb * QW:(vb + 1) * QW], sc_ps[:],
                                        mybir.ActivationFunctionType.Exp,
                                        bias=mskb_sb[:, mcol:mcol + 1], scale=1.0)
                                    nc.tensor.matmul(
                                        rs_ps[:], ones_bf[:],
                                        attnT[:, vb * QW:(vb + 1) * QW],
                                        start=(vb == 0), stop=(vb == 7))
                                rcp_sb = pb.tile([1, QW], F32, tag="rcp", bufs=2)
                                nc.vector.reciprocal(rcp_sb[:], rs_ps[:])
                                ctx_ps = psB.tile([P, QW], F32, tag="ctx",
                                                  bufs=2)
                                for vb in range(8):
                                    nc.tensor.matmul(
                                        ctx_ps[:],
                                        v_sb[:, (b * 8 + vb) * CW + h * P:
                                             (b * 8 + vb) * CW + (h + 1) * P],
                                        attnT[:, vb * QW:(vb + 1) * QW],
                                        start=(vb == 0), stop=(vb == 7))
                                rcp_ps = psB.tile([P, QW], F32, tag="rcpp",
                                                  bufs=1)
                                nc.tensor.matmul(rcp_ps[:], ones_f32[:],
                                                 rcp_sb[:], start=True, stop=True)
                                rcp_rep = pb.tile([P, QW], F32, tag="rcprep",
                                                  bufs=rcprep_bufs)
                                nc.scalar.copy(out=rcp_rep[:], in_=rcp_ps[:])
                                nc.vector.tensor_tensor(
                                    out=ctxT_sb[:, qcol: qcol + QW],
                                    in0=ctx_ps[:], in1=rcp_rep[:],
                                    op=mybir.AluOpType.mult)
                        # after heads {0,1} done, fire first half A2A
                        if do_a2a and h == 1:
                            for hh in range(2):
                                nc.sync.dma_start(
                                    out=a2a_in[0][:, hh * P:(hh + 1) * P, :]
                                        .rearrange("j p l -> p j l"),
                                    in_=ctxT_sb[:, hh * L2:(hh + 1) * L2]
                                        .rearrange("p (j l) -> p j l", j=NCORES))
                            nc.gpsimd.collective_compute(
                                "AllToAll", mybir.AluOpType.bypass,
                                replica_groups=[list(range(NCORES))],
                                ins=[a2a_in[0][:]], outs=[a2a_out[0][:]])
                            for i in range(NCORES):
                                nc.sync.dma_start(
                                    out=octxT[:, (i * NHL) * TL:
                                              (i * NHL + 2) * TL]
                                        .rearrange("p (ct l) -> p ct l", ct=2),
                                    in_=a2a_out[0][i]
                                        .rearrange("(ct p) l -> p ct l", p=P))

                if do_a2a:
                    for hh in range(2):
                        nc.sync.dma_start(
                            out=a2a_in[1][:, hh * P:(hh + 1) * P, :]
                                .rearrange("j p l -> p j l"),
                            in_=ctxT_sb[:, (2 + hh) * L2:(3 + hh) * L2]
                                .rearrange("p (j l) -> p j l", j=NCORES))
                    nc.gpsimd.collective_compute(
                        "AllToAll", mybir.AluOpType.bypass,
                        replica_groups=[list(range(NCORES))],
                        ins=[a2a_in[1][:]], outs=[a2a_out[1][:]])

                # ---------------- Phase C: O-proj, residual + LN ------------
                with tc.tile_pool(name="phaseC", bufs=2) as pc, \
                     tc.tile_pool(name="psC", bufs=4, space="PSUM") as psC:
                    # octxT column layout: g = i*NHL + ct, tile g at cols
                    # [g*TL, (g+1)*TL). Half-0 loads were issued right after
                    # collective #1 inside phase B; load half 1 here.
                    for i in range(NCORES if (do_c and do_a2a) else 0):
                        nc.sync.dma_start(
                            out=octxT[:, (i * NHL + 2) * TL:
                                      (i * NHL + 4) * TL]
                                .rearrange("p (ct l) -> p ct l", ct=2),
                            in_=a2a_out[1][i]
                                .rearrange("(ct p) l -> p ct l", p=P))
                    half0 = [i * NHL + ct for ct in (0, 1)
                             for i in range(NCORES)]
                    half1 = [i * NHL + ct for ct in (2, 3)
                             for i in range(NCORES)]
                    g_order = half0 + half1

                    def oproj_mms(po, wo_sb, lt, gs, start, stop):
                        for g in gs:
                            nc.tensor.matmul(
                                po[:],
                                octxT[:, g * TL + lt * P:
                                      g * TL + (lt + 1) * P],
                                wo_sb[:, g * MCW:(g + 1) * MCW],
                                start=(start and g == gs[0]),
                                stop=(stop and g == gs[-1]))

                    if do_c:
                        x_sb = [big.tile([P, H], BF, tag="big", bufs=5,
                                         name=f"x_sb{lt}") for lt in range(2)]
                        hb_sb = [big.tile([P, H], F32, tag="big", bufs=5,
                                          name=f"hb_sb{lt}") for lt in range(2)]
                        ms8 = [pc.tile([P, MC], F32, tag="ms8",
                                       name=f"ms8{lt}") for lt in range(2)]
                        nc.sync.dma_start(out=g_sb[:], in_=g_d[:])
                        nc.sync.dma_start(out=bta_sb[:], in_=bta_d[:])
                        for lt in range(2):
                            nc.sync.dma_start(
                                out=hb_sb[lt][:],
                                in_=hb_d[lt * P:(lt + 1) * P, :])

                        def drain(po, mc, lt):
                            # x = po + hb, accumulating row-sums for mean
                            nc.vector.scalar_tensor_tensor(
                                out=x_sb[lt][:, mc * MCW:(mc + 1) * MCW],
                                in0=po[:], scalar=1.0,
                                in1=hb_sb[lt][:, mc * MCW:(mc + 1) * MCW],
                                op0=mybir.AluOpType.mult,
                                op1=mybir.AluOpType.add,
                                accum_out=ms8[lt][:, mc:mc + 1])

                        # stage 1: mc 0-1 chains issue their half-0 matmuls
                        # first -- they only need collective #1, so the PE
                        # works while collective #2 is still in flight.
                        po_s1 = {}
                        for mc in range(2):
                            for lt in range(2):
                                po = psC.tile([P, MCW], F32, tag="po",
                                              name=f"po{mc}{lt}")
                                po_s1[mc, lt] = po
                                oproj_mms(po, wo_sbs[mc], lt, half0,
                                          True, False)
                        for mc in range(2):
                            for lt in range(2):
                                po = po_s1[mc, lt]
                                oproj_mms(po, wo_sbs[mc], lt, half1,
                                          False, True)
                                drain(po, mc, lt)
                        # stage 2: remaining chunks, full chains
                        for mc in range(2, MC):
                            wo_sb = pfc.tile([P, KT * MCW], BF, tag="wo",
                                             bufs=2, name=f"wo{mc}")
                            nc.sync.dma_start(
                                out=wo_sb[:],
                                in_=woT_d[:, mc * (KT * MCW):
                                          (mc + 1) * (KT * MCW)])
                            for lt in range(2):
                                po = psC.tile([P, MCW], F32, tag="po")
                                oproj_mms(po, wo_sb, lt, g_order, True, True)
                                drain(po, mc, lt)

                        for lt in range(2):
                            x = x_sb[lt]
                            o_sb = big.tile([P, H], BF, tag="big", bufs=5,
                                            name=f"o_sb{lt}")
                            musum = pc.tile([P, 1], F32, tag="musum")
                            nc.vector.tensor_reduce(
                                musum[:], ms8[lt][:], mybir.AxisListType.X,
                                mybir.AluOpType.add)
                            mu_neg = pc.tile([P, 1], F32, tag="mu")
                            nc.scalar.mul(mu_neg[:], musum[:], -1.0 / H)
                            ssq = pc.tile([P, 1], F32, tag="ssq")
                            nc.scalar.activation(
                                o_sb[:], x[:],
                                mybir.ActivationFunctionType.Square,
                                bias=mu_neg[:], scale=1.0, accum_out=ssq[:])
                            std = pc.tile([P, 1], F32, tag="std")
                            nc.scalar.activation(
                                std[:], ssq[:],
                                mybir.ActivationFunctionType.Sqrt,
                                bias=eps_sb[:], scale=1.0 / H)
                            rstd = pc.tile([P, 1], F32, tag="rstd")
                            nc.vector.reciprocal(rstd[:], std[:])
                            nc.vector.tensor_scalar(
                                out=x[:], in0=x[:], scalar1=mu_neg[:],
                                scalar2=rstd[:], op0=mybir.AluOpType.add,
                                op1=mybir.AluOpType.mult)
                            nc.vector.scalar_tensor_tensor(
                                out=o_sb[:], in0=x[:], scalar=1.0,
                                in1=g_sb[:], op0=mybir.AluOpType.mult,
                                op1=mybir.AluOpType.mult)
                            nc.vector.tensor_tensor(
                                out=o_sb[:], in0=o_sb[:], in1=bta_sb[:],
                                op=mybir.AluOpType.add)
                            nc.sync.dma_start(out=out_d[lt * P:(lt + 1) * P, :],
                                              in_=o_sb[:])
                pfc.release()
                big.release()

    nc.compile()
    return nc


def _prep_inputs(hidden_states, vision_features, attention_mask,
                 Wq, bq, Wk, bk, Wv, bv, Wo, bo, ln_g, ln_b):
    f = np.asarray
    hs = f(hidden_states, dtype=np.float32).reshape(L2, H)
    vf = f(vision_features, dtype=np.float32).reshape(L2, H)
    am = f(attention_mask)
    Wq, bq = f(Wq, dtype=np.float32), f(bq, dtype=np.float32)
    Wk, bk = f(Wk, dtype=np.float32), f(bk, dtype=np.float32)
    Wv, bv = f(Wv, dtype=np.float32), f(bv, dtype=np.float32)
    Wo, bo = f(Wo, dtype=np.float32), f(bo, dtype=np.float32)
    ln_g, ln_b = f(ln_g, dtype=np.float32), f(ln_b, dtype=np.float32)

    s = 1.0 / np.sqrt(P)

    def tile_act(x):  # [L2, H] -> [P, (q, kt, l)] with x[q*QW+l, kt*P+p]
        t = x.reshape(NQ, QW, KT, P).transpose(3, 0, 2, 1)
        return np.ascontiguousarray(t.reshape(P, NQ * KT * QW)).astype(BF16)

    def tile_w(wT):  # [H, CW] -> [P, (kt, c)]
        t = wT.reshape(KT, P, CW).transpose(1, 0, 2)
        return np.ascontiguousarray(t.reshape(P, KT * CW)).astype(BF16)

    hidT = tile_act(hs)
    visT = tile_act(vf)
    WoT = np.ascontiguousarray(Wo.T)  # [H, H]
    wo_t = np.ascontiguousarray(
        WoT.reshape(KT, P, MC, MCW).transpose(1, 2, 0, 3)
        .reshape(P, MC * KT * MCW)).astype(BF16)
    mb = np.where(am != 0, 0.0, MSK).astype(np.float32)          # (B, LB)
    mskb = np.ascontiguousarray(
        mb.reshape(B, 8, P).transpose(2, 0, 1).reshape(P, B * 8))
    bo_eff = bo + Wo @ bv
    g_rep = np.ascontiguousarray(np.broadcast_to(ln_g, (P, H))).astype(BF16)
    b_rep = np.ascontiguousarray(np.broadcast_to(ln_b, (P, H))).astype(BF16)

    in_maps = []
    for c in range(NCORES):
        sl = slice(c * CW, (c + 1) * CW)
        in_maps.append({
            "hidT": hidT,
            "visT": visT,
            "wqT": tile_w(np.ascontiguousarray((Wq[sl] * s).T)),
            "wkT": tile_w(np.ascontiguousarray(Wk[sl].T)),
            "wvT": tile_w(np.ascontiguousarray(Wv[sl].T)),
            "woT": wo_t,
            "bqT": np.ascontiguousarray((bq[sl] * s).reshape(NHL, P).T),
            "bkT": np.ascontiguousarray(bk[sl].reshape(NHL, P).T),
            "mskb": mskb,
            "hb": np.ascontiguousarray(hs[c * TL:(c + 1) * TL] + bo_eff),
            "g": g_rep,
            "bta": b_rep,
        })
    return in_maps


def kernel(**inputs) -> np.ndarray:
    inputs.pop("_debug", None)
    if "main" not in _CACHE:
        _CACHE["main"] = _build(debug=False)
    nc = _CACHE["main"]
    in_maps = _prep_inputs(**inputs)
    res = run_bass_kernel_spmd(nc, in_maps, list(range(NCORES)))
    out = np.concatenate([res.results[c]["out"] for c in range(NCORES)], axis=0)
    return out.reshape(B, LB, H).astype(np.float32)
